# revision 60
# baseline (speedup 1.0000x reference)
"""AvgPool2d(64x64, stride 1, auto_pad-replicate) on TRN2, 8 NeuronCores.

Reference computes, per (n, c) plane X [256, 256]:
    inner = box_sum_64x64(X) / 4096            # [193, 193]
    out[io, jo] = inner[clamp(io-31, 0, 192), clamp(jo-31, 0, 192)]

The sliding-window sums are linear maps:  inner = Bv^T @ X @ Bw  with
constant banded 0/1 matrices [256, 193] (Bw carries the 1/4096 scale).
On the PE array this is two matmul stages with NO transposes:
    stage A: matmul(lhsT=X_chunk   [h,w],  rhs=Bv [h,io]) -> Y^T [w, io]
    stage B: matmul(lhsT=Y^T_chunk [w,io], rhs=Bw [w,jo]) -> inner [io, jo]
(The per-plane data rides as the stationary operand; the band matrices are
the moving operand.)  Only the 193 distinct rows/cols are computed; the
replicate padding back to 256x256 is done on the host (np.pad edge).

I/O is bf16 (products are data*{0,1} with fp32 PSUM accumulation, so the
only rounding is input/intermediate quantization, ~3e-3 rel).  Host packs
x into a partition-major layout [r, plane, k, w] so each DMA reads one
long contiguous run per partition; the output comes back partition-major
too and is unpacked on the host.

Sharding: pure data parallel, batch dim 16 -> 2 per core, 128 (n,c)
planes per core. No collectives.
"""

import ml_dtypes
import numpy as np

import concourse.bass as bass
import concourse.tile as tile
from concourse import mybir
from concourse.bass_utils import run_bass_kernel_spmd


N_CORES = 8
N, C, H, W = 16, 64, 256, 256
KPOOL = 64
PLANES_PER_CORE = (N // N_CORES) * C  # 128
OUT_I = H - KPOOL + 1  # 193 distinct output rows/cols
PAD_LO = (H - OUT_I) // 2  # 31
PAD_HI = H - OUT_I - PAD_LO  # 32

MM_DT = mybir.dt.bfloat16
MM_NP = ml_dtypes.bfloat16
OUT_DT = mybir.dt.bfloat16
OUT_NP = ml_dtypes.bfloat16

# mo2 (single row io=192) PSUM partition by plane index %% 4, placed in the
# half not used by that plane's mo1 block (even planes: mo1 at 0..63, odd:
# 64..127)
MO2_PART = (64, 0, 96, 32)

BATCH = 8  # planes per DMA transfer
QPIPE_BUFS = 10  # y tiles in flight (stage B trails stage A by up to 8+4)


def _band(n: int, k: int, scale: float) -> np.ndarray:
    """B[i, o] = scale if o <= i < o + k else 0;  [n, n-k+1]."""
    m = n - k + 1
    b = np.zeros((n, m), dtype=np.float32)
    for o in range(m):
        b[o : o + k, o] = scale
    return b


def _split_multiwaits(nc: bass.Bass) -> None:
    """Walrus codegen allows a single sync-wait slot per compute instruction.

    Tile's semaphore assignment can emit several; hoist the extras onto
    standalone NOPs (which lower to pure sequencer waits) in front of the
    instruction, on the same engine, preserving order and semantics.
    """
    f = nc.m.functions[0]
    for block in f.blocks:
        out = []
        for inst in block.instructions:
            si = inst.sync_info
            if si is not None and len(si.on_wait) > 1:
                waits = list(si.on_wait)
                for w in waits[:-1]:
                    nop = mybir.InstNoOp(name=f"WS-{nc.next_id()}", ins=[], outs=[])
                    nop.engine = inst.engine
                    nop.sync_info = mybir.SyncInfo(on_wait=[w], on_update=[])
                    out.append(nop)
                inst.sync_info = mybir.SyncInfo(
                    on_wait=[waits[-1]], on_update=list(si.on_update)
                )
            out.append(inst)
        block.instructions = out


def _build(split_waits: bool = True, sim_init: bool = False) -> bass.Bass:
    nc = bass.Bass()
    # partition-major layouts: x [r, plane, k, w], out [r, plane, mo, jo]
    x_ext = nc.declare_dram_parameter(
        "x", [128, PLANES_PER_CORE, 2, W], MM_DT, isOutput=False
    )
    bv_ext = nc.declare_dram_parameter("bv", [H, OUT_I], MM_DT, isOutput=False)
    bw_ext = nc.declare_dram_parameter("bw", [W, OUT_I], MM_DT, isOutput=False)
    out_ext = nc.declare_dram_parameter(
        "out", [128, PLANES_PER_CORE, 2, OUT_I], OUT_DT, isOutput=True
    )

    n_batches = PLANES_PER_CORE // BATCH
    M2 = OUT_I - 128  # 65, second io chunk

    with tile.TileContext(nc) as tc:
        with (
            tc.tile_pool(name="consts", bufs=1) as consts,
            tc.tile_pool(name="xin", bufs=4) as xpool,
            tc.tile_pool(name="ysb", bufs=QPIPE_BUFS) as ypool_sb,
            tc.tile_pool(name="osb", bufs=4) as opool_sb,
            tc.tile_pool(name="yps", bufs=4, space="PSUM") as ypool_ps,
            tc.tile_pool(name="ops", bufs=4, space="PSUM") as opool_ps,
        ):
            # Band matrices, rows split into 2 chunks of 128 partitions:
            # [r, k, o] with global row = 128*k + r.
            bv_sb = consts.tile([128, 2, OUT_I], MM_DT)
            nc.sync.dma_start(
                out=bv_sb, in_=bv_ext[:, :].rearrange("(k r) o -> r k o", k=2)
            )
            bw_sb = consts.tile([128, 2, OUT_I], MM_DT)
            nc.sync.dma_start(
                out=bw_sb, in_=bw_ext[:, :].rearrange("(k r) o -> r k o", k=2)
            )

            x_tiles = [None] * n_batches
            o_tiles = [None] * n_batches
            y_tiles = {}

            def dma_in(b):
                x_tiles[b] = xpool.tile([128, BATCH, 2, W], MM_DT, name="x_sb")
                # smaller leading sub-transfers so plane 0 lands early
                splits = (0, 2, 4, 8) if b == 0 else (0, 8)
                for lo, hi in zip(splits[:-1], splits[1:]):
                    nc.sync.dma_start(
                        out=x_tiles[b][:, lo:hi],
                        in_=x_ext[:, b * BATCH + lo : b * BATCH + hi, :, :],
                    )

            # issue the first input batch immediately after the consts, at the
            # head of the SP stream
            dma_in(0)

            # Keep the PE HAM-warm while the first input DMA is in flight:
            # dummy matmuls on the band matrix into a scratch PSUM bank.
            # borrows an o_ps slot (none are live during warmup)
            warm_ps = opool_ps.tile(
                [128, OUT_I], mybir.dt.float32, name="warm_ps", tag="o_ps"
            )
            for _ in range(41):
                nc.tensor.matmul(
                    warm_ps,
                    lhsT=bv_sb[:, 0, 0:128],
                    rhs=bv_sb[:, 0, :],
                    start=True,
                    stop=True,
                )

            def band_matmuls(out_ps, col0, lhsT_of_k, band_sb):
                for k in range(2):
                    nc.tensor.matmul(
                        out_ps[:, col0 : col0 + OUT_I],
                        lhsT=lhsT_of_k(k),
                        rhs=band_sb[:, k, :],
                        start=(k == 0),
                        stop=(k == 1),
                    )

            def stage_a(i):
                b, p = divmod(i, BATCH)
                if p == 0 and b > 0:
                    dma_in(b)
                x_sb = x_tiles[b]
                y_ps = ypool_ps.tile([128, 2 * OUT_I], mybir.dt.float32)
                for m in range(2):  # w-chunk -> PSUM partitions
                    band_matmuls(
                        y_ps,
                        m * OUT_I,
                        lambda k, m=m: x_sb[:, p, k, m * 128 : (m + 1) * 128],
                        bv_sb,
                    )
                y_sb = ypool_sb.tile([128, 2 * OUT_I], MM_DT)
                nc.vector.tensor_copy(y_sb, y_ps)
                y_tiles[i] = y_sb

            def stage_b_quad(q):
                """Planes 4q..4q+3.  io chunks {128, 64, 1}: the 64-wide
                chunks of plane pairs run concurrently in disjoint PE column
                groups (tile_position), the 1-wide chunks pack 4 planes."""
                planes = [4 * q + t for t in range(4)]
                b = planes[0] // BATCH
                if planes[0] % BATCH == 0:
                    o_tiles[b] = opool_sb.tile(
                        [128, BATCH, 2, OUT_I], OUT_DT, name="o_sb"
                    )
                ys = [y_tiles.pop(i) for i in planes]
                ops = [
                    opool_ps.tile([128, 2 * OUT_I], mybir.dt.float32, name="o_ps")
                    for _ in planes
                ]
                if sim_init:  # keep the race detector happy about stale rows
                    for o_ps in ops:
                        nc.vector.memset(o_ps[:, OUT_I : 2 * OUT_I], 0.0)
                # mo0: io rows 0..127, full-width matmuls
                for t in range(4):
                    band_matmuls(
                        ops[t],
                        0,
                        lambda k, t=t: ys[t][:, k * OUT_I : k * OUT_I + 128],
                        bw_sb,
                    )
                # mo1: io rows 128..191 (64 wide) -- pack plane pairs into
                # column groups (0,0)/(0,64); output partitions match.
                for pair in (0, 2):
                    for k in range(2):
                        for t in (pair, pair + 1):
                            lo = 64 * (t % 2)
                            nc.tensor.matmul(
                                ops[t][lo : lo + 64, OUT_I : 2 * OUT_I],
                                lhsT=ys[t][:, k * OUT_I + 128 : k * OUT_I + 192],
                                rhs=bw_sb[:, k, :],
                                start=(k == 0),
                                stop=(k == 1),
                                tile_position=(0, lo),
                            )
                # mo2: io rows 161..192 (32 wide; only row 192 is new, the
                # rest duplicate mo1 rows and are ignored) -- pack all 4
                # planes into the 4 column groups
                for k in range(2):
                    for t in range(4):
                        lo = MO2_PART[t]
                        nc.tensor.matmul(
                            ops[t][lo : lo + 32, OUT_I : 2 * OUT_I],
                            lhsT=ys[t][:, k * OUT_I + 161 : k * OUT_I + 193],
                            rhs=bw_sb[:, k, :],
                            start=(k == 0),
                            stop=(k == 1),
                            tile_position=(0, lo),
                        )
                for t in range(4):
                    i = planes[t]
                    p = i % BATCH
                    nc.scalar.copy(o_tiles[b][:, p, :, :], ops[t])
                    flush = (3, 5, 7) if b == n_batches - 1 else (3, 7)
                    if p in flush:
                        prev = ([-1] + list(flush))[flush.index(p)] + 1
                        eng = nc.scalar if (b + flush.index(p)) % 2 == 0 else nc.sync
                        eng.dma_start(
                            out=out_ext[:, b * BATCH + prev : b * BATCH + p + 1, :, :],
                            in_=o_tiles[b][:, prev : p + 1],
                        )

            total = PLANES_PER_CORE
            QPIPE = 8  # planes of slack between stage A and the B quads
            for i in range(total + QPIPE):
                if i < total:
                    stage_a(i)
                j = i - QPIPE
                if j >= 0 and j % 4 == 3:
                    stage_b_quad(j // 4)

    if split_waits:
        _split_multiwaits(nc)
    return nc


_NC_CACHE = None


def _get_nc():
    global _NC_CACHE
    if _NC_CACHE is None:
        _NC_CACHE = _build()
    return _NC_CACHE


def _run(x: np.ndarray, trace: bool = False):
    x = np.asarray(x, dtype=np.float32)
    assert x.shape == (N, C, H, W), x.shape
    # partition-major repack: [core, plane, (k r), w] -> [core, r, plane, k, w]
    xs = x.reshape(N_CORES, PLANES_PER_CORE, 2, 128, W).transpose(0, 3, 1, 2, 4)
    xs = np.ascontiguousarray(xs, dtype=np.float32).astype(MM_NP)
    bv = _band(H, KPOOL, 1.0).astype(MM_NP)
    bw = _band(W, KPOOL, 1.0 / (KPOOL * KPOOL)).astype(MM_NP)
    in_maps = [{"x": xs[i], "bv": bv, "bw": bw} for i in range(N_CORES)]
    res = run_bass_kernel_spmd(
        nc=_get_nc(), in_maps=in_maps, core_ids=list(range(N_CORES)), trace=trace
    )
    # unpack: out [r, plane, mo, jo]; io rows 0..127 at mo=0, rows 128..191
    # at mo=1 partitions 64*(plane%2).., row 192 at mo=1 partition
    # MO2_PART[plane%4]
    P = PLANES_PER_CORE
    pidx = np.arange(P)
    mid_rows = ((pidx % 2) * 64)[None, :] + np.arange(64)[:, None]  # [64, P]
    outs = []
    for i in range(N_CORES):
        o = np.asarray(res.results[i]["out"], dtype=np.float32)
        top = o[:, :, 0, :].transpose(1, 0, 2)  # [P, 128, 193]
        m1 = o[:, :, 1, :]  # [128, P, 193]
        mid = m1[mid_rows, pidx[None, :], :].transpose(1, 0, 2)  # [P, 64, 193]
        last = m1[np.asarray(MO2_PART)[pidx % 4] + 31, pidx, :][:, None, :]  # [P, 1, 193]
        outs.append(np.concatenate([top, mid, last], axis=1))
    inner = np.stack(outs, axis=0)  # [cores, planes, 193, 193]
    full = np.pad(
        inner, ((0, 0), (0, 0), (PAD_LO, PAD_HI), (PAD_LO, PAD_HI)), mode="edge"
    )
    return full.reshape(N, C, H, W), res


def kernel(x: np.ndarray) -> np.ndarray:
    out, _ = _run(x, trace=False)
    return out


# revision 68
# speedup vs baseline: 1.2688x; 1.2688x over previous
"""AvgPool2d(64x64, stride 1, auto_pad-replicate) on TRN2, 8 NeuronCores.

Reference computes, per (n, c) plane X [256, 256]:
    inner = box_sum_64x64(X) / 4096            # [193, 193]
    out[io, jo] = inner[clamp(io-31, 0, 192), clamp(jo-31, 0, 192)]

The sliding-window sums are linear maps:  inner = Bv^T @ X @ Bw  with
constant banded 0/1 matrices [256, 193] (Bw carries the 1/4096 scale).
On the PE array this is two matmul stages with NO transposes:
    stage A: matmul(lhsT=X_chunk   [h,w],  rhs=Bv [h,io]) -> Y^T [w, io]
    stage B: matmul(lhsT=Y^T_chunk [w,io], rhs=Bw [w,jo]) -> inner [io, jo]
(The per-plane data rides as the stationary operand; the band matrices are
the moving operand.)  Only the 193 distinct rows/cols are computed; the
replicate padding back to 256x256 is done on the host (np.pad edge).

I/O is bf16 (products are data*{0,1} with fp32 PSUM accumulation, so the
only rounding is input/intermediate quantization, ~3e-3 rel).  Host packs
x into a partition-major layout [r, plane, k, w] so each DMA reads one
long contiguous run per partition; the output comes back partition-major
too and is unpacked on the host.

Sharding: pure data parallel, batch dim 16 -> 2 per core, 128 (n,c)
planes per core. No collectives.
"""

import ml_dtypes
import numpy as np

import concourse.bass as bass
import concourse.tile as tile
from concourse import mybir
from concourse.bass_utils import run_bass_kernel_spmd


N_CORES = 8
N, C, H, W = 16, 64, 256, 256
KPOOL = 64
PLANES_PER_CORE = (N // N_CORES) * C  # 128
OUT_I = H - KPOOL + 1  # 193 distinct output rows/cols
PAD_LO = (H - OUT_I) // 2  # 31
PAD_HI = H - OUT_I - PAD_LO  # 32

MM_DT = mybir.dt.bfloat16
MM_NP = ml_dtypes.bfloat16
OUT_DT = mybir.dt.bfloat16
OUT_NP = ml_dtypes.bfloat16

BATCH = 8  # planes per DMA transfer
PIPE = 2  # software-pipeline distance between stage A and stage B


def _band(n: int, k: int, scale: float) -> np.ndarray:
    """B[i, o] = scale if o <= i < o + k else 0;  [n, n-k+1]."""
    m = n - k + 1
    b = np.zeros((n, m), dtype=np.float32)
    for o in range(m):
        b[o : o + k, o] = scale
    return b


def _split_multiwaits(nc: bass.Bass) -> None:
    """Walrus codegen allows a single sync-wait slot per compute instruction.

    Tile's semaphore assignment can emit several; hoist the extras onto
    standalone NOPs (which lower to pure sequencer waits) in front of the
    instruction, on the same engine, preserving order and semantics.
    """
    f = nc.m.functions[0]
    for block in f.blocks:
        out = []
        for inst in block.instructions:
            si = inst.sync_info
            if si is not None and len(si.on_wait) > 1:
                waits = list(si.on_wait)
                for w in waits[:-1]:
                    nop = mybir.InstNoOp(name=f"WS-{nc.next_id()}", ins=[], outs=[])
                    nop.engine = inst.engine
                    nop.sync_info = mybir.SyncInfo(on_wait=[w], on_update=[])
                    out.append(nop)
                inst.sync_info = mybir.SyncInfo(
                    on_wait=[waits[-1]], on_update=list(si.on_update)
                )
            out.append(inst)
        block.instructions = out


def _build(split_waits: bool = True, sim_init: bool = False) -> bass.Bass:
    nc = bass.Bass()
    # partition-major layouts: x [r, plane, k, w], out [r, plane, mo, jo]
    x_ext = nc.declare_dram_parameter(
        "x", [128, PLANES_PER_CORE, 2, W], MM_DT, isOutput=False
    )
    bv_ext = nc.declare_dram_parameter("bv", [H, OUT_I], MM_DT, isOutput=False)
    bw_ext = nc.declare_dram_parameter("bw", [W, OUT_I], MM_DT, isOutput=False)
    out_ext = nc.declare_dram_parameter(
        "out", [128, PLANES_PER_CORE, 2, OUT_I], OUT_DT, isOutput=True
    )

    n_batches = PLANES_PER_CORE // BATCH
    M2 = OUT_I - 128  # 65, second io chunk

    with tile.TileContext(nc) as tc:
        with (
            tc.tile_pool(name="consts", bufs=1) as consts,
            tc.tile_pool(name="xin", bufs=4) as xpool,
            tc.tile_pool(name="ysb", bufs=PIPE + 4) as ypool_sb,
            tc.tile_pool(name="osb", bufs=5) as opool_sb,
            tc.tile_pool(name="yps", bufs=4, space="PSUM") as ypool_ps,
            tc.tile_pool(name="ops", bufs=4, space="PSUM") as opool_ps,
        ):
            # Band matrices, rows split into 2 chunks of 128 partitions:
            # [r, k, o] with global row = 128*k + r.
            bv_sb = consts.tile([128, 2, OUT_I], MM_DT)
            nc.sync.dma_start(
                out=bv_sb, in_=bv_ext[:, :].rearrange("(k r) o -> r k o", k=2)
            )
            bw_sb = consts.tile([128, 2, OUT_I], MM_DT)
            nc.sync.dma_start(
                out=bw_sb, in_=bw_ext[:, :].rearrange("(k r) o -> r k o", k=2)
            )

            x_tiles = [None] * n_batches
            o_tiles = [None] * n_batches
            y_tiles = {}

            def dma_in(b):
                x_tiles[b] = xpool.tile([128, BATCH, 2, W], MM_DT, name="x_sb")
                # smaller leading sub-transfers so plane 0 lands early
                splits = (0, 2, 4, 8) if b == 0 else (0, 8)
                for lo, hi in zip(splits[:-1], splits[1:]):
                    nc.sync.dma_start(
                        out=x_tiles[b][:, lo:hi],
                        in_=x_ext[:, b * BATCH + lo : b * BATCH + hi, :, :],
                    )

            # issue the first input batch immediately after the consts, at the
            # head of the SP stream
            dma_in(0)

            # Keep the PE HAM-warm while the first input DMA is in flight:
            # dummy matmuls on the band matrix into a scratch PSUM bank.
            # borrows an o_ps slot (none are live during warmup)
            warm_ps = opool_ps.tile(
                [128, OUT_I], mybir.dt.float32, name="warm_ps", tag="o_ps"
            )
            for _ in range(41):
                nc.tensor.matmul(
                    warm_ps,
                    lhsT=bv_sb[:, 0, 0:128],
                    rhs=bv_sb[:, 0, :],
                    start=True,
                    stop=True,
                )

            def band_matmuls(out_ps, col0, lhsT_of_k, band_sb):
                for k in range(2):
                    nc.tensor.matmul(
                        out_ps[:, col0 : col0 + OUT_I],
                        lhsT=lhsT_of_k(k),
                        rhs=band_sb[:, k, :],
                        start=(k == 0),
                        stop=(k == 1),
                    )

            def stage_a(i):
                b, p = divmod(i, BATCH)
                if p == 0 and b > 0:
                    dma_in(b)
                x_sb = x_tiles[b]
                y_ps = ypool_ps.tile([128, 2 * OUT_I], mybir.dt.float32)
                for m in range(2):  # w-chunk -> PSUM partitions
                    band_matmuls(
                        y_ps,
                        m * OUT_I,
                        lambda k, m=m: x_sb[:, p, k, m * 128 : (m + 1) * 128],
                        bv_sb,
                    )
                y_sb = ypool_sb.tile([128, 2 * OUT_I], MM_DT)
                nc.vector.tensor_copy(y_sb, y_ps)
                y_tiles[i] = y_sb

            def stage_b(i):
                b, p = divmod(i, BATCH)
                if p == 0:
                    o_tiles[b] = opool_sb.tile(
                        [128, BATCH, 2, OUT_I], OUT_DT, name="o_sb"
                    )
                y_sb = y_tiles.pop(i)
                o_ps = opool_ps.tile([128, 2 * OUT_I], mybir.dt.float32)
                if sim_init:  # keep the race detector happy about stale rows
                    nc.vector.memset(o_ps[M2:128, OUT_I : 2 * OUT_I], 0.0)
                for mo, mlen in ((0, 128), (1, M2)):  # io chunk -> PSUM partitions
                    band_matmuls(
                        o_ps[:mlen],
                        mo * OUT_I,
                        lambda k, mo=mo, mlen=mlen: y_sb[
                            :, k * OUT_I + mo * 128 : k * OUT_I + mo * 128 + mlen
                        ],
                        bw_sb,
                    )
                # partitions 65..127 of the mo=1 half carry stale PSUM data;
                # the host discards them.
                nc.scalar.copy(o_tiles[b][:, p, :, :], o_ps)
                # flush output every 4 planes, alternating HWDGE rings so
                # the drain spreads across both queues
                flush = (3, 5, 7) if b == n_batches - 1 else (3, 7)
                if p in flush:
                    prev = ([-1] + list(flush))[flush.index(p)] + 1
                    eng = nc.scalar if (b + flush.index(p)) % 2 == 0 else nc.sync
                    eng.dma_start(
                        out=out_ext[:, b * BATCH + prev : b * BATCH + p + 1, :, :],
                        in_=o_tiles[b][:, prev : p + 1],
                    )

            total = PLANES_PER_CORE
            for i in range(total + PIPE):
                if i < total:
                    stage_a(i)
                if i >= PIPE:
                    stage_b(i - PIPE)

    if split_waits:
        _split_multiwaits(nc)
    return nc


_NC_CACHE = None


def _get_nc():
    global _NC_CACHE
    if _NC_CACHE is None:
        _NC_CACHE = _build()
    return _NC_CACHE


def _run(x: np.ndarray, trace: bool = False):
    x = np.asarray(x, dtype=np.float32)
    assert x.shape == (N, C, H, W), x.shape
    # partition-major repack: [core, plane, (k r), w] -> [core, r, plane, k, w]
    xs = x.reshape(N_CORES, PLANES_PER_CORE, 2, 128, W).transpose(0, 3, 1, 2, 4)
    xs = np.ascontiguousarray(xs, dtype=np.float32).astype(MM_NP)
    bv = _band(H, KPOOL, 1.0).astype(MM_NP)
    bw = _band(W, KPOOL, 1.0 / (KPOOL * KPOOL)).astype(MM_NP)
    in_maps = [{"x": xs[i], "bv": bv, "bw": bw} for i in range(N_CORES)]
    res = run_bass_kernel_spmd(
        nc=_get_nc(), in_maps=in_maps, core_ids=list(range(N_CORES)), trace=trace
    )
    # unpack: out [r, plane, mo, jo] -> [plane, mo*128 + r, jo], valid io < 193
    outs = []
    for i in range(N_CORES):
        o = np.asarray(res.results[i]["out"], dtype=np.float32)
        o = o.transpose(1, 2, 0, 3).reshape(PLANES_PER_CORE, 256, OUT_I)[:, :OUT_I, :]
        outs.append(o)
    inner = np.stack(outs, axis=0)  # [cores, planes, 193, 193]
    full = np.pad(
        inner, ((0, 0), (0, 0), (PAD_LO, PAD_HI), (PAD_LO, PAD_HI)), mode="edge"
    )
    return full.reshape(N, C, H, W), res


def kernel(x: np.ndarray) -> np.ndarray:
    out, _ = _run(x, trace=False)
    return out
